# revision 1
# baseline (speedup 1.0000x reference)
"""MoE expert-parallel FFN kernel for TRN2 (8 NeuronCores).

Reference computation (per expert e):
    h = gelu(x_e @ W1[e] + b1[e]);  y_e = h @ W2[e] + b2[e]
with x = inputs[0].reshape(E, CAP, D), E=8, CAP=4096, D=1024, F=4096.

Sharding: expert parallel — core e owns expert e and its CAP-token slice.
No cross-core communication.

Per-core dataflow (all matmuls in float32r = full PE rate, ~1e-4 rel err):
  mm1: hT[f, tok] = W1[d, f].T @ xT[d, tok]   (K=D, lhsT=W1 natural layout)
       fused bias+gelu on psum eviction (ACT, per-partition bias = b1)
  mm2: y[tok, d] = hT[f, tok].T @ W2[f, d]    (K=F, lhsT=hT as produced)
       b2 added by DVE at psum eviction.
The host pre-transposes x to xT and pre-tiles W1 so every DMA is
partition-contiguous.
"""

import sys

if "/opt/trn_rl_repo" not in sys.path:
    sys.path.insert(0, "/opt/trn_rl_repo")

from contextlib import ExitStack

import numpy as np

import concourse.bacc as bacc
import concourse.tile as tile
from concourse import mybir
from concourse.bass_utils import run_bass_kernel_spmd

E, CAP, D, F = 8, 4096, 1024, 4096
P = 128
TC = 1024            # tokens per tile
NT = CAP // TC       # token tiles per core
KD = D // P          # k-tiles for mm1
FM = F // P          # f chunks
TM = TC // P         # token sub-tiles for mm2
NDH = D // 512       # output d halves

F32 = mybir.dt.float32
F32R = mybir.dt.float32r
BF16 = mybir.dt.bfloat16
F16 = mybir.dt.float16
GELU = mybir.ActivationFunctionType.Gelu_apprx_tanh

# fp16 operands: same 1 cycle/row PE rate as f32r/bf16 (cost model), but
# half the DMA bytes + SBUF footprint, and FWL (2x LDWEIGHTS) applies.
# Max-rel error vs fp32 reference ~4e-4 (simulated) vs 2e-2 tolerance.
MM_DT = F16
# V2 mm2 schedule: W2 resident in SBUF (loaded once), mm2 loops token-block
# outer so only NDH psum banks are live per block (smooth rotation instead of
# holding all 8 banks for a whole dh sweep and stalling at phase boundaries).
MM2_TM_OUTER = False  # HW A/B: tm-outer mm2 measured 1146us vs 1074us dh-outer
# V3: keep W1 resident too (requires TC_TOKENS<=512 for SBUF fit); cuts
# steady-state DMA to x+y only.
W1_RESIDENT = False
TC_TOKENS = None  # None -> module default TC
# mm2-swap: stationary=W2 [128f,128d] tile, moving=hT [128f,512tok], output
# yT [d, tok] (host transposes, outside HW time). Gives mm2 the same
# LDW:stream ratio (1:1024) and 2-bank psum rotation as mm1.
MM2_SWAP = False
# V6 deferred gelu: phase-probe showed mm1-only runs at ACT's gelu rate
# (~129.5us/tile measured vs 109us PE work; mm2-only is PE-bound at ~112us).
# Decouple: DVE evicts psum->hpre (f16 SBUF) so PE never waits on ACT;
# ACT applies gelu+bias hpre->ht lagging behind, catching up during mm2.
DEFER_GELU = False
LDW_OPT = False  # walrus redundant-LDW elision wedges the device (NRT_EXEC_UNIT_UNRECOVERABLE)

_cache = {}


def _install_ldw_opt_patch():
    """Flip walrus's --enable-ldw-opt flag (redundant weight-load elision).

    Consecutive matmuls in this kernel share stationary operands; eliding
    the second LDWEIGHTS removes dead weight-load time on the PE.
    """
    import concourse.bass_utils as bu

    if getattr(bu, "_ldw_opt_patched", False):
        return
    orig = bu.run_command

    def patched(cmd, **kw):
        if LDW_OPT and isinstance(cmd, list):
            cmd = ["--enable-ldw-opt=true" if c == "--enable-ldw-opt=false" else c
                   for c in cmd]
        return orig(cmd, **kw)

    bu.run_command = patched
    bu._ldw_opt_patched = True


_install_ldw_opt_patch()


def _build(mm_dt=None, repeat=1, mm1_pair=True, mm2_tm_outer=None,
           w1_resident=False, tc_tokens=None, phase="both", mm2_swap=None,
           defer_gelu=None):
    if mm_dt is None:
        mm_dt = MM_DT
    if mm2_tm_outer is None:
        mm2_tm_outer = MM2_TM_OUTER
    if mm2_swap is None:
        mm2_swap = MM2_SWAP
    if defer_gelu is None:
        defer_gelu = DEFER_GELU
    # local tiling (module-level TC/NT/TM describe the default config)
    TCl = TC if tc_tokens is None else tc_tokens
    NTl = CAP // TCl
    TMl = TCl // P
    assert not (w1_resident and TCl > 512), "W1+W2+ht don't fit SBUF at TC>512"
    nc = bacc.Bacc("TRN2", target_bir_lowering=False, debug=False)

    xt = nc.dram_tensor("xt", [D, CAP], mm_dt, kind="ExternalInput")
    w1t = nc.dram_tensor("w1t", [FM, P, KD, P], mm_dt, kind="ExternalInput")
    w2 = nc.dram_tensor("w2", [F, D], mm_dt, kind="ExternalInput")
    b1t = nc.dram_tensor("b1t", [P, FM], F32, kind="ExternalInput")
    b2b = nc.dram_tensor("b2b", [P, D], F32, kind="ExternalInput")
    if mm2_swap:
        b2c = nc.dram_tensor("b2c", [P, D // P], F32, kind="ExternalInput")
        # host-retiled W2: w2s[dm][p, fm, q] = W2[fm*P+p, dm*P+q], so each
        # d-block's stationary set is one fully-contiguous DMA
        w2s = nc.dram_tensor("w2s", [D // P, P, FM, P], mm_dt,
                             kind="ExternalInput")
        y = nc.dram_tensor("y", [D, CAP], F32, kind="ExternalOutput")
    else:
        y = nc.dram_tensor("y", [CAP, D], F32, kind="ExternalOutput")

    with tile.TileContext(nc) as tc:
        with ExitStack() as ctx:
            const = ctx.enter_context(tc.tile_pool(name="const", bufs=1))
            xpool = ctx.enter_context(tc.tile_pool(name="x", bufs=1))
            htpool = ctx.enter_context(tc.tile_pool(name="ht", bufs=1))
            w1pool = ctx.enter_context(tc.tile_pool(name="w1", bufs=3))
            w2pool = ctx.enter_context(tc.tile_pool(name="w2", bufs=8))
            ypool = ctx.enter_context(tc.tile_pool(name="yev", bufs=4))
            psum = ctx.enter_context(tc.tile_pool(name="psum", bufs=8, space="PSUM"))
            hppool = None
            if defer_gelu:
                # pre-gelu staging chunks; sized to absorb ACT's worst-case
                # backlog (~32 chunks) without stalling the DVE evictions
                hppool = ctx.enter_context(tc.tile_pool(name="hp", bufs=40))

            b1_sb = const.tile([P, FM], F32, name=f"b1_sb_ldw{int(LDW_OPT)}")
            nc.sync.dma_start(b1_sb[:], b1t.ap())
            b2_sb = const.tile([P, D], F32)
            nc.sync.dma_start(b2_sb[:], b2b.ap())
            b2c_sb = None
            if mm2_swap:
                b2c_sb = const.tile([P, D // P], F32)
                nc.sync.dma_start(b2c_sb[:], b2c.ap())

            xt_r = xt.ap().rearrange("(k p) c -> p k c", p=P)  # [128, KD, CAP]
            w1_r = w1t.ap()  # [FM, P, KD, P]
            w2_r = w2.ap()
            y_r = y.ap()

            w2res_sb = None
            w2_r4 = None
            if mm2_tm_outer:
                # whole W2 resident: [128 f-part, FM, D]. Loaded once, but the
                # chunk DMAs are issued inside the first tile's mm1 fm-loop so
                # they queue BEHIND the x/w1 chunks the first matmuls need
                # (issuing them here would delay PE start by ~16us).
                w2res_sb = const.tile([P, FM, D], mm_dt)
                w2_r4 = w2.ap().rearrange("(fm p) d -> fm p d", p=P)
            w1res_sb = None
            if w1_resident:
                # whole W1 resident: [128 d-part, FM, KD, 128 f]; chunk DMAs
                # interleaved into the first tile's fm loop like w2res (except
                # fm=0, needed immediately, issued before the x chunks).
                w1res_sb = const.tile([P, FM, KD, P], mm_dt)

            ht_fill = None
            if phase == "mm2":
                # mm2-only probe: pre-fill ht once with normal-range data
                ht_fill = htpool.tile([P, FM, TCl], mm_dt, tag="ht")
                for fm in range(FM):
                    nc.sync.dma_start(ht_fill[:, fm], xt_r[:, fm % KD, 0:TCl])

            for it, t in enumerate(
                [t for _ in range(repeat) for t in range(NTl)]
            ):
                first = it == 0
                if phase == "mm2":
                    pass
                elif not w1_resident:
                    # w1[fm=0] issued BEFORE the x chunks so the first matmul
                    # group isn't queued behind the whole x tile
                    w1_next = w1pool.tile([P, KD, P], mm_dt, tag="w1", name="w1p")
                    nc.sync.dma_start(w1_next[:], w1_r[0])
                elif first:
                    nc.sync.dma_start(w1res_sb[:, 0], w1_r[0])

                if phase != "mm2":
                    x_sb = xpool.tile([P, KD, TCl], mm_dt, tag="x")
                    # per-(k, h) chunk DMAs, k-major: matches matmul
                    # consumption order and interleaves with weight-stream
                    # DMAs so a single monolithic transfer can't starve the
                    # weight queues
                    for k in range(KD):
                        for h in range(TCl // 512):
                            nc.sync.dma_start(
                                x_sb[:, k, h * 512:(h + 1) * 512],
                                xt_r[:, k,
                                     t * TCl + h * 512:t * TCl + (h + 1) * 512],
                            )

                ht_sb = (ht_fill if phase == "mm2"
                         else htpool.tile([P, FM, TCl], mm_dt, tag="ht"))

                # --- mm1: hT[f_chunk, tok] += W1.T @ xT, fused bias+gelu ---
                # k-outer with both token-half psums live: consecutive matmul
                # pairs share the stationary w1 slice (redundant-LDW elision)
                NH = TCl // 512
                for fm in (range(FM) if phase != "mm2" else ()):
                    if w1_resident:
                        def w1_slice(k, fm=fm):
                            return w1res_sb[:, fm, k]
                        if first and fm + 1 < FM:
                            nc.sync.dma_start(w1res_sb[:, fm + 1], w1_r[fm + 1])
                    else:
                        w1_sb = w1_next
                        if fm + 1 < FM:
                            w1_next = w1pool.tile([P, KD, P], mm_dt, tag="w1",
                                                  name="w1p")
                            nc.sync.dma_start(w1_next[:], w1_r[fm + 1])

                        def w1_slice(k, w1_sb=w1_sb):
                            return w1_sb[:, k]
                    if mm2_tm_outer and first:
                        nc.sync.dma_start(w2res_sb[:, fm], w2_r4[fm])
                    if mm1_pair:
                        ps_h = [
                            psum.tile([P, 512], F32, tag="ps", name="psh")
                            for _ in range(NH)
                        ]
                        for k in range(KD):
                            for h in range(NH):
                                nc.tensor.matmul(
                                    ps_h[h][:],
                                    w1_slice(k),
                                    x_sb[:, k, h * 512:(h + 1) * 512],
                                    start=(k == 0),
                                    stop=(k == KD - 1),
                                )
                        for h in range(NH):
                            if defer_gelu:
                                hp = hppool.tile([P, 512], mm_dt, tag="hp")
                                nc.vector.tensor_copy(hp[:], ps_h[h][:])
                                nc.scalar.activation(
                                    ht_sb[:, fm, h * 512:(h + 1) * 512],
                                    hp[:],
                                    GELU,
                                    bias=b1_sb[:, fm:fm + 1],
                                )
                            else:
                                nc.scalar.activation(
                                    ht_sb[:, fm, h * 512:(h + 1) * 512],
                                    ps_h[h][:],
                                    GELU,
                                    bias=b1_sb[:, fm:fm + 1],
                                )
                    else:
                        for h in range(NH):
                            ps = psum.tile([P, 512], F32, tag="ps", name="psh")
                            for k in range(KD):
                                nc.tensor.matmul(
                                    ps[:],
                                    w1_slice(k),
                                    x_sb[:, k, h * 512:(h + 1) * 512],
                                    start=(k == 0),
                                    stop=(k == KD - 1),
                                )
                            nc.scalar.activation(
                                ht_sb[:, fm, h * 512:(h + 1) * 512],
                                ps[:],
                                GELU,
                                bias=b1_sb[:, fm:fm + 1],
                            )

                if phase == "mm1":
                    continue

                if mm2_swap:
                    # --- mm2-swap: yT[d, tok] += W2.T-tile @ hT ---
                    # stationary = W2 [128f, 128d] slices from a per-d-block
                    # tile (one contiguous 1MB DMA each, prefetched one
                    # block ahead), moving = hT [128f, 512tok]; 1 LDW per
                    # 1024 stream cycles, 2 psum banks live per d-block.
                    NDM = D // P
                    w2s_r = w2s.ap()
                    w2dm_next = w2pool.tile([P, FM, P], mm_dt, tag="w2")
                    nc.sync.dma_start(w2dm_next[:], w2s_r[0])
                    for dm in range(NDM):
                        w2dm = w2dm_next
                        if dm + 1 < NDM:
                            w2dm_next = w2pool.tile([P, FM, P], mm_dt,
                                                    tag="w2")
                            nc.sync.dma_start(w2dm_next[:], w2s_r[dm + 1])
                        ps_w = [
                            psum.tile([P, 512], F32, tag="ps", name="psw")
                            for _ in range(NH)
                        ]
                        for fm in range(FM):
                            for h in range(NH):
                                nc.tensor.matmul(
                                    ps_w[h][:],
                                    w2dm[:, fm],
                                    ht_sb[:, fm, h * 512:(h + 1) * 512],
                                    start=(fm == 0),
                                    stop=(fm == FM - 1),
                                )
                        for h in range(NH):
                            y_sb = ypool.tile([P, 512], F32, tag="y")
                            nc.scalar.activation(
                                y_sb[:], ps_w[h][:],
                                mybir.ActivationFunctionType.Identity,
                                bias=b2c_sb[:, dm:dm + 1],
                            )
                            nc.sync.dma_start(
                                y_r[dm * P:(dm + 1) * P,
                                    t * TCl + h * 512:t * TCl + (h + 1) * 512],
                                y_sb[:],
                            )
                    continue

                # --- mm2: y[tok, d] += hT.T @ W2, b2 added on eviction ---
                if mm2_tm_outer:
                    # token-block outer: per tm only NDH psum banks live; their
                    # eviction overlaps the next tm's 2*FM matmuls. W2 comes
                    # from the resident SBUF copy (no DMA in the loop).
                    for tm in range(TMl):
                        ps_y = [
                            psum.tile([P, 512], F32, tag="ps", name="psy")
                            for _ in range(NDH)
                        ]
                        for fm in range(FM):
                            for dh in range(NDH):
                                nc.tensor.matmul(
                                    ps_y[dh][:],
                                    ht_sb[:, fm, tm * P:(tm + 1) * P],
                                    w2res_sb[:, fm, dh * 512:(dh + 1) * 512],
                                    start=(fm == 0),
                                    stop=(fm == FM - 1),
                                )
                        for dh in range(NDH):
                            y_sb = ypool.tile([P, 512], F32, tag="y")
                            nc.vector.tensor_add(
                                y_sb[:], ps_y[dh][:],
                                b2_sb[:, dh * 512:(dh + 1) * 512],
                            )
                            nc.sync.dma_start(
                                y_r[t * TCl + tm * P:t * TCl + (tm + 1) * P,
                                    dh * 512:(dh + 1) * 512],
                                y_sb[:],
                            )
                    continue
                for dh in range(NDH):
                    ps_y = [
                        psum.tile([P, 512], F32, tag="ps", name="psy")
                        for i in range(TMl)
                    ]
                    for fm in range(FM):
                        w2_sb = w2pool.tile([P, 512], mm_dt, tag="w2")
                        nc.sync.dma_start(
                            w2_sb[:],
                            w2_r[fm * P:(fm + 1) * P, dh * 512:(dh + 1) * 512],
                        )
                        for tm in range(TMl):
                            nc.tensor.matmul(
                                ps_y[tm][:],
                                ht_sb[:, fm, tm * P:(tm + 1) * P],
                                w2_sb[:],
                                start=(fm == 0),
                                stop=(fm == FM - 1),
                            )
                    for tm in range(TMl):
                        y_sb = ypool.tile([P, 512], F32, tag="y")
                        nc.vector.tensor_add(
                            y_sb[:], ps_y[tm][:], b2_sb[:, dh * 512:(dh + 1) * 512]
                        )
                        nc.sync.dma_start(
                            y_r[t * TCl + tm * P:t * TCl + (tm + 1) * P,
                                dh * 512:(dh + 1) * 512],
                            y_sb[:],
                        )

            if phase == "mm1":
                # probe-only: touch the output once so it exists
                y_sb = ypool.tile([P, 512], F32, tag="y")
                nc.vector.tensor_copy(y_sb[:], b2_sb[:, 0:512])
                nc.sync.dma_start(y_r[0:P, 0:512], y_sb[:])

    nc.compile()
    return nc


def _wire_np_dtype(mm_dt):
    if mm_dt == BF16:
        import ml_dtypes

        return ml_dtypes.bfloat16
    if mm_dt == F16:
        return np.float16
    return np.float32


def _prep_core_inputs(inputs, W1, b1, W2, b2, e, wdt):
    x_e = inputs[0, e * CAP:(e + 1) * CAP, :]          # [CAP, D]
    xt = np.ascontiguousarray(x_e.T).astype(wdt)       # [D, CAP]
    # W1[e]: [D, F] -> [FM, P(d-part... see kernel), KD, P]
    # kernel reads w1t[fm][p, k, f] == W1[k*P + p, fm*P + f]
    w1t = np.ascontiguousarray(
        W1[e].reshape(KD, P, FM, P).transpose(2, 1, 0, 3)
    ).astype(wdt)
    b1t = np.ascontiguousarray(b1[e].reshape(FM, P).T)  # [P, FM]
    b2b = np.ascontiguousarray(np.broadcast_to(b2[e], (P, D)))
    b2c = np.ascontiguousarray(b2[e].reshape(D // P, P).T)  # [P, D/P]
    w2s = np.ascontiguousarray(
        W2[e].reshape(FM, P, D // P, P).transpose(2, 1, 0, 3)
    ).astype(wdt)
    return {
        "xt": xt,
        "w1t": w1t,
        "w2": np.ascontiguousarray(W2[e]).astype(wdt),
        "b1t": b1t,
        "b2b": b2b,
        "b2c": b2c,
        "w2s": w2s,
    }


def get_nc(mm_dt=None, repeat=1, mm1_pair=True, mm2_tm_outer=None,
           w1_resident=None, tc_tokens=None, mm2_swap=None, defer_gelu=None):
    if mm_dt is None:
        mm_dt = MM_DT
    if mm2_tm_outer is None:
        mm2_tm_outer = MM2_TM_OUTER
    if w1_resident is None:
        w1_resident = W1_RESIDENT
    if tc_tokens is None:
        tc_tokens = TC_TOKENS
    if mm2_swap is None:
        mm2_swap = MM2_SWAP
    if defer_gelu is None:
        defer_gelu = DEFER_GELU
    key = (mm_dt, repeat, mm1_pair, mm2_tm_outer, w1_resident, tc_tokens,
           mm2_swap, defer_gelu)
    if key not in _cache:
        _cache[key] = _build(mm_dt, repeat, mm1_pair, mm2_tm_outer,
                             w1_resident, tc_tokens, mm2_swap=mm2_swap,
                             defer_gelu=defer_gelu)
    return _cache[key]


def make_in_maps(inputs, W1, b1, W2, b2, mm_dt=None):
    inputs = np.asarray(inputs, dtype=np.float32)
    W1 = np.asarray(W1, dtype=np.float32)
    b1 = np.asarray(b1, dtype=np.float32)
    W2 = np.asarray(W2, dtype=np.float32)
    b2 = np.asarray(b2, dtype=np.float32)
    wdt = _wire_np_dtype(mm_dt if mm_dt is not None else MM_DT)
    return [_prep_core_inputs(inputs, W1, b1, W2, b2, e, wdt) for e in range(E)]


def kernel(inputs, W1, b1, W2, b2):
    nc = get_nc()
    in_maps = make_in_maps(inputs, W1, b1, W2, b2)
    # The axon-tunneled devices occasionally come up wedged from a previous
    # process (NRT_EXEC_UNIT_UNRECOVERABLE); a backend reset + retry recovers.
    last_err = None
    for attempt in range(3):
        try:
            res = run_bass_kernel_spmd(nc, in_maps, list(range(E))).results
            break
        except Exception as err:  # noqa: BLE001
            last_err = err
            import time as _time

            try:
                import jax as _jax
                import jax.extend.backend as _jxb

                _jax.clear_caches()
                _jxb.clear_backends()
            except Exception:  # noqa: BLE001
                pass
            _time.sleep(10.0 * (attempt + 1))
    else:
        raise last_err
    out = np.empty((1, E * CAP, D), dtype=np.float32)
    for e in range(E):
        ye = res[e]["y"]
        if ye.shape == (D, CAP):  # mm2_swap kernels produce yT
            ye = ye.T
        out[0, e * CAP:(e + 1) * CAP, :] = ye
    return out


if __name__ == "__main__":
    rng = np.random.default_rng(0)
    ins = {
        "inputs": rng.standard_normal((1, E * CAP, D), dtype=np.float32),
        "W1": rng.standard_normal((E, D, F), dtype=np.float32) / np.sqrt(D),
        "b1": np.zeros((E, F), np.float32),
        "W2": rng.standard_normal((E, F, D), dtype=np.float32) / np.sqrt(F),
        "b2": np.zeros((E, D), np.float32),
    }
    y = kernel(**ins)
    print("out", y.shape, y.dtype, float(np.abs(y).mean()))



# revision 8
# speedup vs baseline: 1.0228x; 1.0228x over previous
"""MoE expert-parallel FFN kernel for TRN2 (8 NeuronCores).

Reference computation (per expert e):
    h = gelu(x_e @ W1[e] + b1[e]);  y_e = h @ W2[e] + b2[e]
with x = inputs[0].reshape(E, CAP, D), E=8, CAP=4096, D=1024, F=4096.

Sharding: expert parallel — core e owns expert e and its CAP-token slice.
No cross-core communication.

Per-core dataflow (all matmuls in float32r = full PE rate, ~1e-4 rel err):
  mm1: hT[f, tok] = W1[d, f].T @ xT[d, tok]   (K=D, lhsT=W1 natural layout)
       fused bias+gelu on psum eviction (ACT, per-partition bias = b1)
  mm2: y[tok, d] = hT[f, tok].T @ W2[f, d]    (K=F, lhsT=hT as produced)
       b2 added by DVE at psum eviction.
The host pre-transposes x to xT and pre-tiles W1 so every DMA is
partition-contiguous.
"""

import sys

if "/opt/trn_rl_repo" not in sys.path:
    sys.path.insert(0, "/opt/trn_rl_repo")

from contextlib import ExitStack

import numpy as np

import concourse.bacc as bacc
import concourse.tile as tile
from concourse import mybir
from concourse.bass_utils import run_bass_kernel_spmd

E, CAP, D, F = 8, 4096, 1024, 4096
P = 128
TC = 1024            # tokens per tile
NT = CAP // TC       # token tiles per core
KD = D // P          # k-tiles for mm1
FM = F // P          # f chunks
TM = TC // P         # token sub-tiles for mm2
NDH = D // 512       # output d halves

F32 = mybir.dt.float32
F32R = mybir.dt.float32r
BF16 = mybir.dt.bfloat16
F16 = mybir.dt.float16
GELU = mybir.ActivationFunctionType.Gelu_apprx_tanh

# fp16 operands: same 1 cycle/row PE rate as f32r/bf16 (cost model), but
# half the DMA bytes + SBUF footprint, and FWL (2x LDWEIGHTS) applies.
# Max-rel error vs fp32 reference ~4e-4 (simulated) vs 2e-2 tolerance.
MM_DT = F16
# V2 mm2 schedule: W2 resident in SBUF (loaded once), mm2 loops token-block
# outer so only NDH psum banks are live per block (smooth rotation instead of
# holding all 8 banks for a whole dh sweep and stalling at phase boundaries).
MM2_TM_OUTER = False  # HW A/B: tm-outer mm2 measured 1146us vs 1074us dh-outer
# V3: keep W1 resident too (requires TC_TOKENS<=512 for SBUF fit); cuts
# steady-state DMA to x+y only.
W1_RESIDENT = False
TC_TOKENS = None  # None -> module default TC
# mm2-swap: stationary=W2 [128f,128d] tile, moving=hT [128f,512tok], output
# yT [d, tok] (host transposes, outside HW time). Gives mm2 the same
# LDW:stream ratio (1:1024) and 2-bank psum rotation as mm1.
MM2_SWAP = False
# V7 deferred+batched gelu: phase-probe showed mm1-only runs at ACT's gelu
# rate (~129.5us/tile measured vs 109us PE work ~= 2.02us per [128,512] chunk
# vs ~0.72us modeled -> ~1.3us/instr HW overhead). Fix both ends: DVE evicts
# psum->hp (f16 SBUF) so PE never waits on ACT, and ACT applies gelu over
# ACT_SPAN*TC-column spans (4x fewer instrs at span 2). Spans covering >1 fm
# can't carry the per-fm b1 bias in one instruction -> only used when b1==0
# (true for the reference: b1 = jnp.zeros); kernel() falls back to span=1
# (single-fm spans, bias on ACT) for nonzero b1.
ACT_SPAN = 2  # None -> baseline fused ACT eviction; 1/2/4 -> defer+batch
LDW_OPT = False  # walrus redundant-LDW elision wedges the device (NRT_EXEC_UNIT_UNRECOVERABLE)

_cache = {}


def _install_ldw_opt_patch():
    """Flip walrus's --enable-ldw-opt flag (redundant weight-load elision).

    Consecutive matmuls in this kernel share stationary operands; eliding
    the second LDWEIGHTS removes dead weight-load time on the PE.
    """
    import concourse.bass_utils as bu

    if getattr(bu, "_ldw_opt_patched", False):
        return
    orig = bu.run_command

    def patched(cmd, **kw):
        if LDW_OPT and isinstance(cmd, list):
            cmd = ["--enable-ldw-opt=true" if c == "--enable-ldw-opt=false" else c
                   for c in cmd]
        return orig(cmd, **kw)

    bu.run_command = patched
    bu._ldw_opt_patched = True


_install_ldw_opt_patch()


def _build(mm_dt=None, repeat=1, mm1_pair=True, mm2_tm_outer=None,
           w1_resident=False, tc_tokens=None, phase="both", mm2_swap=None,
           act_span=None):
    if mm_dt is None:
        mm_dt = MM_DT
    if mm2_tm_outer is None:
        mm2_tm_outer = MM2_TM_OUTER
    if mm2_swap is None:
        mm2_swap = MM2_SWAP
    if act_span is None:
        act_span = ACT_SPAN
    # local tiling (module-level TC/NT/TM describe the default config)
    TCl = TC if tc_tokens is None else tc_tokens
    NTl = CAP // TCl
    TMl = TCl // P
    assert not (w1_resident and TCl > 512), "W1+W2+ht don't fit SBUF at TC>512"
    nc = bacc.Bacc("TRN2", target_bir_lowering=False, debug=False)

    xt = nc.dram_tensor("xt", [D, CAP], mm_dt, kind="ExternalInput")
    w1t = nc.dram_tensor("w1t", [FM, P, KD, P], mm_dt, kind="ExternalInput")
    w2 = nc.dram_tensor("w2", [F, D], mm_dt, kind="ExternalInput")
    b1t = nc.dram_tensor("b1t", [P, FM], F32, kind="ExternalInput")
    b2b = nc.dram_tensor("b2b", [P, D], F32, kind="ExternalInput")
    if mm2_swap:
        b2c = nc.dram_tensor("b2c", [P, D // P], F32, kind="ExternalInput")
        # host-retiled W2: w2s[dm][p, fm, q] = W2[fm*P+p, dm*P+q], so each
        # d-block's stationary set is one fully-contiguous DMA
        w2s = nc.dram_tensor("w2s", [D // P, P, FM, P], mm_dt,
                             kind="ExternalInput")
        y = nc.dram_tensor("y", [D, CAP], F32, kind="ExternalOutput")
    else:
        y = nc.dram_tensor("y", [CAP, D], F32, kind="ExternalOutput")

    with tile.TileContext(nc) as tc:
        with ExitStack() as ctx:
            const = ctx.enter_context(tc.tile_pool(name="const", bufs=1))
            xpool = ctx.enter_context(tc.tile_pool(name="x", bufs=1))
            htpool = ctx.enter_context(tc.tile_pool(name="ht", bufs=1))
            w1pool = ctx.enter_context(tc.tile_pool(name="w1", bufs=3))
            w2pool = ctx.enter_context(tc.tile_pool(name="w2", bufs=8))
            ypool = ctx.enter_context(tc.tile_pool(name="yev", bufs=4))
            psum = ctx.enter_context(tc.tile_pool(name="psum", bufs=8, space="PSUM"))
            hppool = None
            if act_span:
                # pre-gelu staging spans ([P, act_span*TC] f16); ring deep
                # enough that DVE evictions never wait on ACT's gelu pass
                hppool = ctx.enter_context(tc.tile_pool(name="hp", bufs=4))

            b1_sb = const.tile([P, FM], F32, name=f"b1_sb_ldw{int(LDW_OPT)}")
            nc.sync.dma_start(b1_sb[:], b1t.ap())
            b2_sb = const.tile([P, D], F32)
            nc.sync.dma_start(b2_sb[:], b2b.ap())
            b2c_sb = None
            if mm2_swap:
                b2c_sb = const.tile([P, D // P], F32)
                nc.sync.dma_start(b2c_sb[:], b2c.ap())

            xt_r = xt.ap().rearrange("(k p) c -> p k c", p=P)  # [128, KD, CAP]
            w1_r = w1t.ap()  # [FM, P, KD, P]
            w2_r = w2.ap()
            y_r = y.ap()

            w2res_sb = None
            w2_r4 = None
            if mm2_tm_outer:
                # whole W2 resident: [128 f-part, FM, D]. Loaded once, but the
                # chunk DMAs are issued inside the first tile's mm1 fm-loop so
                # they queue BEHIND the x/w1 chunks the first matmuls need
                # (issuing them here would delay PE start by ~16us).
                w2res_sb = const.tile([P, FM, D], mm_dt)
                w2_r4 = w2.ap().rearrange("(fm p) d -> fm p d", p=P)
            w1res_sb = None
            if w1_resident:
                # whole W1 resident: [128 d-part, FM, KD, 128 f]; chunk DMAs
                # interleaved into the first tile's fm loop like w2res (except
                # fm=0, needed immediately, issued before the x chunks).
                w1res_sb = const.tile([P, FM, KD, P], mm_dt)

            ht_fill = None
            if phase == "mm2":
                # mm2-only probe: pre-fill ht once with normal-range data
                ht_fill = htpool.tile([P, FM, TCl], mm_dt, tag="ht")
                for fm in range(FM):
                    nc.sync.dma_start(ht_fill[:, fm], xt_r[:, fm % KD, 0:TCl])

            for it, t in enumerate(
                [t for _ in range(repeat) for t in range(NTl)]
            ):
                first = it == 0
                if phase == "mm2":
                    pass
                elif not w1_resident:
                    # w1[fm=0] issued BEFORE the x chunks so the first matmul
                    # group isn't queued behind the whole x tile
                    w1_next = w1pool.tile([P, KD, P], mm_dt, tag="w1", name="w1p")
                    nc.sync.dma_start(w1_next[:], w1_r[0])
                elif first:
                    nc.sync.dma_start(w1res_sb[:, 0], w1_r[0])

                if phase != "mm2":
                    x_sb = xpool.tile([P, KD, TCl], mm_dt, tag="x")
                    # per-(k, h) chunk DMAs, k-major: matches matmul
                    # consumption order and interleaves with weight-stream
                    # DMAs so a single monolithic transfer can't starve the
                    # weight queues
                    for k in range(KD):
                        for h in range(TCl // 512):
                            nc.sync.dma_start(
                                x_sb[:, k, h * 512:(h + 1) * 512],
                                xt_r[:, k,
                                     t * TCl + h * 512:t * TCl + (h + 1) * 512],
                            )

                ht_sb = (ht_fill if phase == "mm2"
                         else htpool.tile([P, FM, TCl], mm_dt, tag="ht"))

                # --- mm1: hT[f_chunk, tok] += W1.T @ xT, fused bias+gelu ---
                # k-outer with both token-half psums live: consecutive matmul
                # pairs share the stationary w1 slice (redundant-LDW elision)
                NH = TCl // 512
                for fm in (range(FM) if phase != "mm2" else ()):
                    if w1_resident:
                        def w1_slice(k, fm=fm):
                            return w1res_sb[:, fm, k]
                        if first and fm + 1 < FM:
                            nc.sync.dma_start(w1res_sb[:, fm + 1], w1_r[fm + 1])
                    else:
                        w1_sb = w1_next
                        if fm + 1 < FM:
                            w1_next = w1pool.tile([P, KD, P], mm_dt, tag="w1",
                                                  name="w1p")
                            nc.sync.dma_start(w1_next[:], w1_r[fm + 1])

                        def w1_slice(k, w1_sb=w1_sb):
                            return w1_sb[:, k]
                    if mm2_tm_outer and first:
                        nc.sync.dma_start(w2res_sb[:, fm], w2_r4[fm])
                    if mm1_pair:
                        if act_span and fm % act_span == 0:
                            hp = hppool.tile(
                                [P, act_span * NH * 512], mm_dt, tag="hp",
                                name="hp",
                            )
                        ps_h = [
                            psum.tile([P, 512], F32, tag="ps", name="psh")
                            for _ in range(NH)
                        ]
                        for k in range(KD):
                            for h in range(NH):
                                nc.tensor.matmul(
                                    ps_h[h][:],
                                    w1_slice(k),
                                    x_sb[:, k, h * 512:(h + 1) * 512],
                                    start=(k == 0),
                                    stop=(k == KD - 1),
                                )
                        if act_span:
                            j = fm % act_span
                            for h in range(NH):
                                nc.vector.tensor_copy(
                                    hp[:, (j * NH + h) * 512:
                                       (j * NH + h + 1) * 512],
                                    ps_h[h][:],
                                )
                            if j == act_span - 1:
                                fm0 = fm - act_span + 1
                                if act_span == 1:
                                    nc.scalar.activation(
                                        ht_sb[:, fm, :],
                                        hp[:],
                                        GELU,
                                        bias=b1_sb[:, fm:fm + 1],
                                    )
                                else:
                                    # span covers act_span fm rows: per-fm
                                    # bias impossible -> requires b1 == 0
                                    nc.scalar.activation(
                                        ht_sb[:, fm0:fm + 1, :],
                                        hp[:],
                                        GELU,
                                    )
                        else:
                            for h in range(NH):
                                nc.scalar.activation(
                                    ht_sb[:, fm, h * 512:(h + 1) * 512],
                                    ps_h[h][:],
                                    GELU,
                                    bias=b1_sb[:, fm:fm + 1],
                                )
                    else:
                        for h in range(NH):
                            ps = psum.tile([P, 512], F32, tag="ps", name="psh")
                            for k in range(KD):
                                nc.tensor.matmul(
                                    ps[:],
                                    w1_slice(k),
                                    x_sb[:, k, h * 512:(h + 1) * 512],
                                    start=(k == 0),
                                    stop=(k == KD - 1),
                                )
                            nc.scalar.activation(
                                ht_sb[:, fm, h * 512:(h + 1) * 512],
                                ps[:],
                                GELU,
                                bias=b1_sb[:, fm:fm + 1],
                            )

                if phase == "mm1":
                    continue

                if mm2_swap:
                    # --- mm2-swap: yT[d, tok] += W2.T-tile @ hT ---
                    # stationary = W2 [128f, 128d] slices from a per-d-block
                    # tile (one contiguous 1MB DMA each, prefetched one
                    # block ahead), moving = hT [128f, 512tok]; 1 LDW per
                    # 1024 stream cycles, 2 psum banks live per d-block.
                    NDM = D // P
                    w2s_r = w2s.ap()
                    w2dm_next = w2pool.tile([P, FM, P], mm_dt, tag="w2")
                    nc.sync.dma_start(w2dm_next[:], w2s_r[0])
                    for dm in range(NDM):
                        w2dm = w2dm_next
                        if dm + 1 < NDM:
                            w2dm_next = w2pool.tile([P, FM, P], mm_dt,
                                                    tag="w2")
                            nc.sync.dma_start(w2dm_next[:], w2s_r[dm + 1])
                        ps_w = [
                            psum.tile([P, 512], F32, tag="ps", name="psw")
                            for _ in range(NH)
                        ]
                        for fm in range(FM):
                            for h in range(NH):
                                nc.tensor.matmul(
                                    ps_w[h][:],
                                    w2dm[:, fm],
                                    ht_sb[:, fm, h * 512:(h + 1) * 512],
                                    start=(fm == 0),
                                    stop=(fm == FM - 1),
                                )
                        for h in range(NH):
                            y_sb = ypool.tile([P, 512], F32, tag="y")
                            nc.scalar.activation(
                                y_sb[:], ps_w[h][:],
                                mybir.ActivationFunctionType.Identity,
                                bias=b2c_sb[:, dm:dm + 1],
                            )
                            nc.sync.dma_start(
                                y_r[dm * P:(dm + 1) * P,
                                    t * TCl + h * 512:t * TCl + (h + 1) * 512],
                                y_sb[:],
                            )
                    continue

                # --- mm2: y[tok, d] += hT.T @ W2, b2 added on eviction ---
                if mm2_tm_outer:
                    # token-block outer: per tm only NDH psum banks live; their
                    # eviction overlaps the next tm's 2*FM matmuls. W2 comes
                    # from the resident SBUF copy (no DMA in the loop).
                    for tm in range(TMl):
                        ps_y = [
                            psum.tile([P, 512], F32, tag="ps", name="psy")
                            for _ in range(NDH)
                        ]
                        for fm in range(FM):
                            for dh in range(NDH):
                                nc.tensor.matmul(
                                    ps_y[dh][:],
                                    ht_sb[:, fm, tm * P:(tm + 1) * P],
                                    w2res_sb[:, fm, dh * 512:(dh + 1) * 512],
                                    start=(fm == 0),
                                    stop=(fm == FM - 1),
                                )
                        for dh in range(NDH):
                            y_sb = ypool.tile([P, 512], F32, tag="y")
                            nc.vector.tensor_add(
                                y_sb[:], ps_y[dh][:],
                                b2_sb[:, dh * 512:(dh + 1) * 512],
                            )
                            nc.sync.dma_start(
                                y_r[t * TCl + tm * P:t * TCl + (tm + 1) * P,
                                    dh * 512:(dh + 1) * 512],
                                y_sb[:],
                            )
                    continue
                for dh in range(NDH):
                    ps_y = [
                        psum.tile([P, 512], F32, tag="ps", name="psy")
                        for i in range(TMl)
                    ]
                    for fm in range(FM):
                        w2_sb = w2pool.tile([P, 512], mm_dt, tag="w2")
                        nc.sync.dma_start(
                            w2_sb[:],
                            w2_r[fm * P:(fm + 1) * P, dh * 512:(dh + 1) * 512],
                        )
                        for tm in range(TMl):
                            nc.tensor.matmul(
                                ps_y[tm][:],
                                ht_sb[:, fm, tm * P:(tm + 1) * P],
                                w2_sb[:],
                                start=(fm == 0),
                                stop=(fm == FM - 1),
                            )
                    for tm in range(TMl):
                        y_sb = ypool.tile([P, 512], F32, tag="y")
                        nc.vector.tensor_add(
                            y_sb[:], ps_y[tm][:], b2_sb[:, dh * 512:(dh + 1) * 512]
                        )
                        nc.sync.dma_start(
                            y_r[t * TCl + tm * P:t * TCl + (tm + 1) * P,
                                dh * 512:(dh + 1) * 512],
                            y_sb[:],
                        )

            if phase == "mm1":
                # probe-only: touch the output once so it exists
                y_sb = ypool.tile([P, 512], F32, tag="y")
                nc.vector.tensor_copy(y_sb[:], b2_sb[:, 0:512])
                nc.sync.dma_start(y_r[0:P, 0:512], y_sb[:])

    nc.compile()
    return nc


def _wire_np_dtype(mm_dt):
    if mm_dt == BF16:
        import ml_dtypes

        return ml_dtypes.bfloat16
    if mm_dt == F16:
        return np.float16
    return np.float32


def _prep_core_inputs(inputs, W1, b1, W2, b2, e, wdt):
    x_e = inputs[0, e * CAP:(e + 1) * CAP, :]          # [CAP, D]
    xt = np.ascontiguousarray(x_e.T).astype(wdt)       # [D, CAP]
    # W1[e]: [D, F] -> [FM, P(d-part... see kernel), KD, P]
    # kernel reads w1t[fm][p, k, f] == W1[k*P + p, fm*P + f]
    w1t = np.ascontiguousarray(
        W1[e].reshape(KD, P, FM, P).transpose(2, 1, 0, 3)
    ).astype(wdt)
    b1t = np.ascontiguousarray(b1[e].reshape(FM, P).T)  # [P, FM]
    b2b = np.ascontiguousarray(np.broadcast_to(b2[e], (P, D)))
    b2c = np.ascontiguousarray(b2[e].reshape(D // P, P).T)  # [P, D/P]
    w2s = np.ascontiguousarray(
        W2[e].reshape(FM, P, D // P, P).transpose(2, 1, 0, 3)
    ).astype(wdt)
    return {
        "xt": xt,
        "w1t": w1t,
        "w2": np.ascontiguousarray(W2[e]).astype(wdt),
        "b1t": b1t,
        "b2b": b2b,
        "b2c": b2c,
        "w2s": w2s,
    }


def get_nc(mm_dt=None, repeat=1, mm1_pair=True, mm2_tm_outer=None,
           w1_resident=None, tc_tokens=None, mm2_swap=None, act_span=None,
           phase="both"):
    if mm_dt is None:
        mm_dt = MM_DT
    if mm2_tm_outer is None:
        mm2_tm_outer = MM2_TM_OUTER
    if w1_resident is None:
        w1_resident = W1_RESIDENT
    if tc_tokens is None:
        tc_tokens = TC_TOKENS
    if mm2_swap is None:
        mm2_swap = MM2_SWAP
    if act_span is None:
        act_span = ACT_SPAN
    key = (mm_dt, repeat, mm1_pair, mm2_tm_outer, w1_resident, tc_tokens,
           mm2_swap, act_span, phase)
    if key not in _cache:
        _cache[key] = _build(mm_dt, repeat, mm1_pair, mm2_tm_outer,
                             w1_resident, tc_tokens, mm2_swap=mm2_swap,
                             act_span=act_span, phase=phase)
    return _cache[key]


def make_in_maps(inputs, W1, b1, W2, b2, mm_dt=None):
    inputs = np.asarray(inputs, dtype=np.float32)
    W1 = np.asarray(W1, dtype=np.float32)
    b1 = np.asarray(b1, dtype=np.float32)
    W2 = np.asarray(W2, dtype=np.float32)
    b2 = np.asarray(b2, dtype=np.float32)
    wdt = _wire_np_dtype(mm_dt if mm_dt is not None else MM_DT)
    return [_prep_core_inputs(inputs, W1, b1, W2, b2, e, wdt) for e in range(E)]


def kernel(inputs, W1, b1, W2, b2):
    span = ACT_SPAN
    if span is not None and span > 1 and np.any(np.asarray(b1)):
        span = 1  # multi-fm gelu spans can't carry a per-fm bias
    nc = get_nc(act_span=span)
    in_maps = make_in_maps(inputs, W1, b1, W2, b2)
    # The axon-tunneled devices occasionally come up wedged from a previous
    # process (NRT_EXEC_UNIT_UNRECOVERABLE); a backend reset + retry recovers.
    last_err = None
    for attempt in range(3):
        try:
            res = run_bass_kernel_spmd(nc, in_maps, list(range(E))).results
            break
        except Exception as err:  # noqa: BLE001
            last_err = err
            import time as _time

            try:
                import jax as _jax
                import jax.extend.backend as _jxb

                _jax.clear_caches()
                _jxb.clear_backends()
            except Exception:  # noqa: BLE001
                pass
            _time.sleep(10.0 * (attempt + 1))
    else:
        raise last_err
    out = np.empty((1, E * CAP, D), dtype=np.float32)
    for e in range(E):
        ye = res[e]["y"]
        if ye.shape == (D, CAP):  # mm2_swap kernels produce yT
            ye = ye.T
        out[0, e * CAP:(e + 1) * CAP, :] = ye
    return out


if __name__ == "__main__":
    rng = np.random.default_rng(0)
    ins = {
        "inputs": rng.standard_normal((1, E * CAP, D), dtype=np.float32),
        "W1": rng.standard_normal((E, D, F), dtype=np.float32) / np.sqrt(D),
        "b1": np.zeros((E, F), np.float32),
        "W2": rng.standard_normal((E, F, D), dtype=np.float32) / np.sqrt(F),
        "b2": np.zeros((E, D), np.float32),
    }
    y = kernel(**ins)
    print("out", y.shape, y.dtype, float(np.abs(y).mean()))

